# revision 1
# baseline (speedup 1.0000x reference)
"""PointNet2 classifier Trainium2 kernel.

Contract: kernel(**inputs) takes full unsharded inputs (x [32,6,4096] plus
weights), shards batch over 8 neuron cores (4 batches/core), runs a Bass
program per core, returns [32,40] fp32.

Per-core program (per batch):
  SA1: scores s=2*q.x - ys (fp32 PE matmul, zero topk flips vs reference),
       DVE top-32 (max/max_index/match_replace rounds), A-matrix trick:
       A = W1@[xyz;pts] stored hi/lo fp16 (22-bit), dma_gather(transpose),
       h1 = relu(Ahi+Alo - C + b1), convs 64->64->128 in fp32r, fused
       max-over-k from PSUM.
  SA2: same with N=512,P=128,k=64, A2 channels 128 (hi/lo 512B rows).
  SA3 + FC head: batched over the 4 batches (512 cols).
"""
import sys
sys.path.insert(0, '/opt/trn_rl_repo')
import numpy as np
from concourse import bass, bacc, tile, bass_utils
from concourse.alu_op_type import AluOpType

mybir = bass.mybir
dt = mybir.dt
AF = mybir.ActivationFunctionType
AX = mybir.AxisListType

BPC = 4
N = 4096
NEG = -3.0e38
NCORES = 8

_cache = {}
last_exec_time_ns = None


def build_program():
    nc = bacc.Bacc("TRN2", target_bir_lowering=False, debug=False,
                   num_devices=1)

    x_d = nc.dram_tensor("x", [BPC, 6, N], dt.float32, kind="ExternalInput")
    ones_d = nc.dram_tensor("ones", [1, 512], dt.float32,
                            kind="ExternalInput")

    def win(name, shape):
        return nc.dram_tensor(name, shape, dt.float32, kind="ExternalInput")

    w1t6_d = win("w1t6", [6, 64])
    w1x1_d = win("w1x1", [3, 64])
    b1_1_d = win("b1_1", [64, 1])
    w2t1_d = win("w2t1", [64, 64])
    b2_1_d = win("b2_1", [64, 1])
    w3t1_d = win("w3t1", [64, 128])
    b3_1_d = win("b3_1", [128, 1])
    w1x2_d = win("w1x2", [3, 128])
    w1p2_d = win("w1p2", [128, 128])
    b1_2_d = win("b1_2", [128, 1])
    w2t2_d = win("w2t2", [128, 128])
    b2_2_d = win("b2_2", [128, 1])
    w3t2_d = win("w3t2", [128, 256])
    b3_2_d = win("b3_2", [128, 2])
    w1p3_d = win("w1p3", [256, 256])
    b1_3_d = win("b1_3", [128, 2])
    w2t3_d = win("w2t3", [256, 512])
    b2_3_d = win("b2_3", [128, 4])
    w3t3_d = win("w3t3", [512, 1024])
    b3_3_d = win("b3_3", [128, 8])
    f1w_d = win("f1w", [1024, 512])
    f1b_d = win("f1b", [128, 4])
    f2w_d = win("f2w", [512, 256])
    f2b_d = win("f2b", [128, 2])
    f3w_d = win("f3w", [256, 40])
    f3b_d = win("f3b", [40, 1])

    eye_d = nc.dram_tensor("eye", [128, 128], dt.float32,
                           kind="ExternalInput")
    eyestk_d = nc.dram_tensor("eyestk", [128, 64], dt.float16,
                              kind="ExternalInput")
    out_d = nc.dram_tensor("out", [BPC, 40], dt.float32,
                           kind="ExternalOutput")

    with tile.TileContext(nc) as tc, \
         tc.tile_pool(name="persist", bufs=1) as pp:
        # ---------------- persistent weight tiles ----------------
        w1t6_t = pp.tile([6, 64], dt.float32r, name="w1t6_t")
        w1x1_t = pp.tile([3, 64], dt.float32, name="w1x1_t")
        b1_1t = pp.tile([64, 1], dt.float32, name="b1_1t")
        w2t1_t = pp.tile([64, 64], dt.float32r, name="w2t1_t")
        b2_1t = pp.tile([64, 1], dt.float32, name="b2_1t")
        w3t1_t = pp.tile([64, 128], dt.float32r, name="w3t1_t")
        b3_1t = pp.tile([128, 1], dt.float32, name="b3_1t")
        w1x2_r = pp.tile([3, 128], dt.float32r, name="w1x2_r")
        w1x2_f = pp.tile([3, 128], dt.float32, name="w1x2_f")
        w1p2_t = pp.tile([128, 128], dt.float32r, name="w1p2_t")
        b1_2t = pp.tile([128, 1], dt.float32, name="b1_2t")
        w2t2_t = pp.tile([128, 128], dt.float32r, name="w2t2_t")
        b2_2t = pp.tile([128, 1], dt.float32, name="b2_2t")
        w3t2_t = pp.tile([128, 256], dt.float32r, name="w3t2_t")
        b3_2t = pp.tile([128, 2], dt.float32, name="b3_2t")
        w1p3_t = [pp.tile([128, 256], dt.float32r, name=f"w1p3_{k}")
                  for k in range(2)]
        b1_3t = pp.tile([128, 2], dt.float32, name="b1_3t")
        w2t3_t = [pp.tile([128, 512], dt.float32r, name=f"w2t3_{k}")
                  for k in range(2)]
        b2_3t = pp.tile([128, 4], dt.float32, name="b2_3t")
        b3_3t = pp.tile([128, 8], dt.float32, name="b3_3t")
        f1bt = pp.tile([128, 4], dt.float32, name="f1bt")
        f2bt = pp.tile([128, 2], dt.float32, name="f2bt")
        f3w_t = [pp.tile([128, 40], dt.float32r, name=f"f3w_{k}")
                 for k in range(2)]
        f3bt = pp.tile([40, 1], dt.float32, name="f3bt")
        ones_t = pp.tile([1, 512], dt.float32, name="ones_t")
        ones3_t = pp.tile([3, 1], dt.float32, name="ones3_t")
        eye_t = pp.tile([128, 128], dt.float32, name="eye_t")
        eyestk_t = pp.tile([128, 64], dt.float16, name="eyestk_t")

        dma = nc.gpsimd.dma_start
        dma(w1t6_t[:], w1t6_d[:])
        dma(w1x1_t[:], w1x1_d[:])
        dma(b1_1t[:], b1_1_d[:])
        dma(w2t1_t[:], w2t1_d[:])
        dma(b2_1t[:], b2_1_d[:])
        dma(w3t1_t[:], w3t1_d[:])
        dma(b3_1t[:], b3_1_d[:])
        dma(w1x2_r[:], w1x2_d[:])
        dma(w1x2_f[:], w1x2_d[:])
        dma(w1p2_t[:], w1p2_d[:])
        dma(b1_2t[:], b1_2_d[:])
        dma(w2t2_t[:], w2t2_d[:])
        dma(b2_2t[:], b2_2_d[:])
        dma(w3t2_t[:], w3t2_d[:])
        dma(b3_2t[:], b3_2_d[:])
        for k in range(2):
            dma(w1p3_t[k][:], w1p3_d[k * 128:(k + 1) * 128, :])
            dma(w2t3_t[k][:], w2t3_d[k * 128:(k + 1) * 128, :])
            dma(f3w_t[k][:], f3w_d[k * 128:(k + 1) * 128, :])
        dma(b1_3t[:], b1_3_d[:])
        dma(b2_3t[:], b2_3_d[:])
        dma(b3_3t[:], b3_3_d[:])
        dma(f1bt[:], f1b_d[:])
        dma(f2bt[:], f2b_d[:])
        dma(f3bt[:], f3b_d[:])
        dma(ones_t[:], ones_d[:])
        dma(ones3_t[:], ones_d[0:1, 0:3])
        dma(eye_t[:], eye_d[:])
        dma(eyestk_t[:], eyestk_d[:])

        # ---------------- persistent working tiles ----------------
        caug = pp.tile([4, N], dt.float32, name="caug")   # [x;y;z;-ys]
        qaug = pp.tile([4, 512], dt.float32, name="qaug")  # [2x;2y;2z;1]
        xyz1_r = pp.tile([3, 512], dt.float32r, name="xyz1_r")
        C1 = pp.tile([64, 512], dt.float32, name="C1")
        C2 = pp.tile([128, 128], dt.float32, name="C2")
        p1pre = pp.tile([128, 512], dt.float32, name="p1pre")
        p1 = pp.tile([128, 512], dt.float32r, name="p1")
        p2pre_h = pp.tile([128, 128], dt.float32, name="p2pre_h")
        p2pre_l = pp.tile([128, 128], dt.float32, name="p2pre_l")
        p2all_h = pp.tile([128, 512], dt.float32r, name="p2all_h")
        p2all_l = pp.tile([128, 512], dt.float32r, name="p2all_l")
        A1sb = pp.tile([64, 4096], dt.float32, name="A1sb")
        A2sb = pp.tile([128, 512], dt.float32, name="A2sb")
        idxw = pp.tile([128, 1024], dt.int16, name="idxw")
        S2 = pp.tile([128, 512], dt.float32, name="S2")
        gall = pp.tile([128, 32], dt.float32r, name="gall")
        hf1 = pp.tile([128, 16], dt.float32r, name="hf1")
        hf2 = pp.tile([128, 8], dt.float32r, name="hf2")
        outsb = pp.tile([40, 4], dt.float32, name="outsb")

        with tc.tile_pool(name="ps", bufs=8, space="PSUM") as ps, \
             tc.tile_pool(name="spool", bufs=2) as spool, \
             tc.tile_pool(name="apool", bufs=3) as apool, \
             tc.tile_pool(name="vpool", bufs=2) as vpool, \
             tc.tile_pool(name="ipool", bufs=2) as ipool, \
             tc.tile_pool(name="hpool", bufs=2) as hpool, \
             tc.tile_pool(name="wpool", bufs=4) as wpool, \
             tc.tile_pool(name="gpool", bufs=2) as gpool:

            for b in range(BPC):
                # ---- load xyz / features ----
                dma(caug[0:3, :], x_d[b, 0:3, :])
                dma(qaug[3:4, :], ones_d[:])
                dma(xyz1_r[:], x_d[b, 0:3, 0:512])

                # ---- squares and ys (caug row 3 = -ys) ----
                for ns in range(8):
                    sqc = vpool.tile([3, 512], dt.float32, name="sqc")
                    nc.scalar.activation(sqc[:],
                                         caug[0:3, ns * 512:(ns + 1) * 512],
                                         AF.Square)
                    ys_ps = ps.tile([1, 512], dt.float32, name="acc")
                    nc.tensor.matmul(ys_ps[:], ones3_t[:], sqc[:],
                                     start=True, stop=True)
                    ys_sb = vpool.tile([1, 512], dt.float32, name="ys_sb")
                    nc.scalar.activation(ys_sb[:], ys_ps[:], AF.Copy,
                                         scale=-1.0)
                    dma(caug[3:4, ns * 512:(ns + 1) * 512], ys_sb[:])
                nc.scalar.activation(qaug[0:3, :], caug[0:3, 0:512],
                                     AF.Copy, scale=2.0)

                # ---- A1 = W1 @ [xyz;pts], channel-major fp32 [64, 4096] ----
                for cg in range(8):
                    xc = apool.tile([6, 512], dt.float32r, name="xc")
                    dma(xc[:], x_d[b, :, cg * 512:(cg + 1) * 512])
                    a_ps = ps.tile([64, 512], dt.float32, name="acc")
                    nc.tensor.matmul(a_ps[:], w1t6_t[:], xc[:],
                                     start=True, stop=True)
                    nc.scalar.activation(A1sb[:, cg * 512:(cg + 1) * 512],
                                         a_ps[:], AF.Copy)

                # ---- C1 = W1[:, :3] @ q_xyz (fp32) ----
                c_ps = ps.tile([64, 512], dt.float32, name="acc")
                nc.tensor.matmul(c_ps[:], w1x1_t[:], caug[0:3, 0:512],
                                 start=True, stop=True)
                nc.scalar.activation(C1[:], c_ps[:], AF.Copy)

                # ---- SA1 scores + top-32 per 128-query chunk ----
                for qc in range(4):
                    S = spool.tile([128, N], dt.float32, name="S")
                    for ns in range(8):
                        s_ps = ps.tile([128, 512], dt.float32, name="acc")
                        nc.tensor.matmul(
                            s_ps[:], qaug[:, qc * 128:(qc + 1) * 128],
                            caug[:, ns * 512:(ns + 1) * 512],
                            start=True, stop=True)
                        nc.scalar.activation(S[:, ns * 512:(ns + 1) * 512],
                                             s_ps[:], AF.Copy)
                    idx32 = ipool.tile([128, 32], dt.uint32, name="idx32")
                    for r in range(4):
                        v8 = vpool.tile([128, 8], dt.float32, name="v8")
                        nc.vector.max(v8[:], S[:])
                        nc.vector.max_index(idx32[:, r * 8:(r + 1) * 8],
                                            v8[:], S[:])
                        if r < 3:
                            nc.vector.match_replace(S[:], v8[:], S[:], NEG)
                    idxf = ipool.tile([128, 32], dt.float32, name="idxf")
                    nc.vector.tensor_copy(idxf[:], idx32[:])
                    tp_ps = ps.tile([32, 128], dt.float32, name="acc")
                    nc.tensor.transpose(tp_ps[:], idxf[:], eye_t[:])
                    tpf = ipool.tile([32, 128], dt.float32, name="tpf")
                    nc.scalar.activation(tpf[:], tp_ps[:], AF.Copy)
                    # wrap: flat i = q*32+r -> idxw[r%16, 2q + r//16]
                    for h in range(2):
                        sel_ps = ps.tile([16, 128], dt.float32, name="acc")
                        nc.tensor.matmul(
                            sel_ps[:], eye_t[0:32, 16 * h:16 * h + 16],
                            tpf[:], start=True, stop=True)
                        dst = idxw[0:16, qc * 256:(qc + 1) * 256].rearrange(
                            "p (q h) -> p q h", h=2)[:, :, h:h + 1]
                        nc.vector.tensor_copy(dst, sel_ps[:].unsqueeze(2))

                # ---- replicate wrap channels 16 -> 128, gather A1 rows ----
                dma(idxw[16:32, 0:1024], idxw[0:16, 0:1024])
                dma(idxw[32:64, 0:1024], idxw[0:32, 0:1024])
                dma(idxw[64:128, 0:1024], idxw[0:64, 0:1024])
                # ---- SA1: per-qc ap_gather + conv (8 chunks of 512) ----
                for qc in range(4):
                    G1f = gpool.tile([64, 4096], dt.float32, name="G1f")
                    nc.gpsimd.ap_gather(
                        G1f[:].unsqueeze(2), A1sb[:].unsqueeze(2),
                        idxw[0:64, qc * 256:(qc + 1) * 256],
                        channels=64, num_elems=4096, d=1, num_idxs=4096)
                    for ch in range(8):
                        cs = slice(ch * 512, (ch + 1) * 512)
                        q0, q1 = qc * 128 + ch * 16, qc * 128 + ch * 16 + 16
                        h1s = hpool.tile([64, 512], dt.float32, name="h1s")
                        cb = C1[:, q0:q1].unsqueeze(2).broadcast_to(
                            [64, 16, 32])
                        nc.vector.tensor_sub(
                            h1s[:].rearrange("p (q k) -> p q k", k=32),
                            G1f[:, cs].rearrange("p (q k) -> p q k", k=32),
                            cb)
                        h1 = hpool.tile([64, 512], dt.float32r, name="h1")
                        nc.scalar.activation(h1[:], h1s[:], AF.Relu,
                                             bias=b1_1t[:])
                        p2_ps = ps.tile([64, 512], dt.float32, name="acc")
                        nc.tensor.matmul(p2_ps[:], w2t1_t[:], h1[:],
                                         start=True, stop=True)
                        h2 = hpool.tile([64, 512], dt.float32r, name="h2")
                        nc.scalar.activation(h2[:], p2_ps[:], AF.Relu,
                                             bias=b2_1t[:])
                        p3_ps = ps.tile([128, 512], dt.float32, name="acc")
                        nc.tensor.matmul(p3_ps[:], w3t1_t[:], h2[:],
                                         start=True, stop=True)
                        nc.vector.tensor_reduce(
                            p1pre[:, q0:q1],
                            p3_ps[:].rearrange("p (q k) -> p q k", k=32),
                            axis=AX.X, op=AluOpType.max)
                nc.scalar.activation(p1[:], p1pre[:], AF.Identity,
                                     bias=b3_1t[:])

                # ---- A2 = W1_2 @ [xyz1;p1], channel-major fp32 [128, 512] --
                a2_ps = ps.tile([128, 512], dt.float32, name="acc")
                nc.tensor.matmul(a2_ps[:], w1x2_r[:], xyz1_r[:],
                                 start=True, stop=False)
                nc.tensor.matmul(a2_ps[:], w1p2_t[:], p1[:],
                                 start=False, stop=True)
                nc.scalar.activation(A2sb[:], a2_ps[:], AF.Copy)

                # ---- C2 (fp32) ----
                c2_ps = ps.tile([128, 128], dt.float32, name="acc")
                nc.tensor.matmul(c2_ps[:], w1x2_f[:], caug[0:3, 0:128],
                                 start=True, stop=True)
                nc.scalar.activation(C2[:], c2_ps[:], AF.Copy)

                # ---- SA2 scores (reuse qaug/caug slices) + top-64 ----
                s2_ps = ps.tile([128, 512], dt.float32, name="acc")
                nc.tensor.matmul(s2_ps[:], qaug[:, 0:128], caug[:, 0:512],
                                 start=True, stop=True)
                nc.scalar.activation(S2[:, 0:512], s2_ps[:], AF.Copy)
                idx64 = ipool.tile([128, 64], dt.uint32, name="idx64")
                for r in range(8):
                    v8 = vpool.tile([128, 8], dt.float32, name="v8b")
                    nc.vector.max(v8[:], S2[:, 0:512])
                    nc.vector.max_index(idx64[:, r * 8:(r + 1) * 8], v8[:],
                                        S2[:, 0:512])
                    if r < 7:
                        nc.vector.match_replace(S2[:, 0:512], v8[:],
                                                S2[:, 0:512], NEG)
                idxf2 = ipool.tile([128, 64], dt.float32, name="idxf2")
                nc.vector.tensor_copy(idxf2[:], idx64[:])
                tp2_ps = ps.tile([64, 128], dt.float32, name="acc")
                nc.tensor.transpose(tp2_ps[:], idxf2[:], eye_t[:])
                tpf2 = ipool.tile([64, 128], dt.float32, name="tpf2")
                nc.scalar.activation(tpf2[:], tp2_ps[:], AF.Copy)
                # wrap: flat i = q*64+r -> idxw[r%16, 4q + r//16]
                for h in range(4):
                    sel_ps = ps.tile([16, 128], dt.float32, name="acc")
                    nc.tensor.matmul(
                        sel_ps[:], eye_t[0:64, 16 * h:16 * h + 16],
                        tpf2[:], start=True, stop=True)
                    dst = idxw[0:16, 0:512].rearrange(
                        "p (q h) -> p q h", h=4)[:, :, h:h + 1]
                    nc.vector.tensor_copy(dst, sel_ps[:].unsqueeze(2))
                dma(idxw[16:32, 0:512], idxw[0:16, 0:512])
                dma(idxw[32:64, 0:512], idxw[0:32, 0:512])
                dma(idxw[64:128, 0:512], idxw[0:64, 0:512])
                # ---- SA2: chunked ap_gather + conv (4x4 chunks of 512) ----
                for c4 in range(4):
                    G2f = gpool.tile([128, 2048], dt.float32, name="G2f")
                    nc.gpsimd.ap_gather(
                        G2f[:].unsqueeze(2), A2sb[:].unsqueeze(2),
                        idxw[0:128, c4 * 128:(c4 + 1) * 128],
                        channels=128, num_elems=512, d=1, num_idxs=2048)
                    for ci in range(4):
                        cs = slice(ci * 512, (ci + 1) * 512)
                        q0 = c4 * 32 + ci * 8
                        q1 = q0 + 8
                        h1s = hpool.tile([128, 512], dt.float32, name="h1s")
                        cb = C2[:, q0:q1].unsqueeze(2).broadcast_to(
                            [128, 8, 64])
                        nc.vector.tensor_sub(
                            h1s[:].rearrange("p (q k) -> p q k", k=64),
                            G2f[:, cs].rearrange("p (q k) -> p q k", k=64),
                            cb)
                        h1 = hpool.tile([128, 512], dt.float32r, name="h1")
                        nc.scalar.activation(h1[:], h1s[:], AF.Relu,
                                             bias=b1_2t[:])
                        p2_ps = ps.tile([128, 512], dt.float32, name="acc")
                        nc.tensor.matmul(p2_ps[:], w2t2_t[:], h1[:],
                                         start=True, stop=True)
                        h2 = hpool.tile([128, 512], dt.float32r, name="h2")
                        nc.scalar.activation(h2[:], p2_ps[:], AF.Relu,
                                             bias=b2_2t[:])
                        ph = ps.tile([128, 512], dt.float32, name="acc")
                        nc.tensor.matmul(ph[:], w3t2_t[:, 0:128], h2[:],
                                         start=True, stop=True)
                        nc.vector.tensor_reduce(
                            p2pre_h[:, q0:q1],
                            ph[:].rearrange("p (q k) -> p q k", k=64),
                            axis=AX.X, op=AluOpType.max)
                        pl = ps.tile([128, 512], dt.float32, name="acc")
                        nc.tensor.matmul(pl[:], w3t2_t[:, 128:256], h2[:],
                                         start=True, stop=True)
                        nc.vector.tensor_reduce(
                            p2pre_l[:, q0:q1],
                            pl[:].rearrange("p (q k) -> p q k", k=64),
                            axis=AX.X, op=AluOpType.max)
                bs = slice(b * 128, (b + 1) * 128)
                nc.scalar.activation(p2all_h[:, bs], p2pre_h[:], AF.Identity,
                                     bias=b3_2t[:, 0:1])
                nc.scalar.activation(p2all_l[:, bs], p2pre_l[:], AF.Identity,
                                     bias=b3_2t[:, 1:2])

            # ---------------- SA3 + FC head (batched, 512 cols) ----------
            h1_3 = [pp.tile([128, 512], dt.float32r, name=f"h1_3{k}")
                    for k in range(2)]
            h2_3 = [pp.tile([128, 512], dt.float32r, name=f"h2_3{k}")
                    for k in range(4)]
            for mc in range(2):
                ms = slice(mc * 128, (mc + 1) * 128)
                px = ps.tile([128, 512], dt.float32, name="acc")
                nc.tensor.matmul(px[:], w1p3_t[0][:, ms], p2all_h[:],
                                 start=True, stop=False)
                nc.tensor.matmul(px[:], w1p3_t[1][:, ms], p2all_l[:],
                                 start=False, stop=True)
                nc.scalar.activation(h1_3[mc][:], px[:], AF.Relu,
                                     bias=b1_3t[:, mc:mc + 1])
            for mc in range(4):
                ms = slice(mc * 128, (mc + 1) * 128)
                px = ps.tile([128, 512], dt.float32, name="acc")
                for kc in range(2):
                    nc.tensor.matmul(px[:], w2t3_t[kc][:, ms], h1_3[kc][:],
                                     start=(kc == 0), stop=(kc == 1))
                nc.scalar.activation(h2_3[mc][:], px[:], AF.Relu,
                                     bias=b2_3t[:, mc:mc + 1])
            for mc in range(8):
                ms = slice(mc * 128, (mc + 1) * 128)
                px = ps.tile([128, 512], dt.float32, name="acc")
                for kc in range(4):
                    wsl = wpool.tile([128, 128], dt.float32r, name="wsl")
                    dma(wsl[:], w3t3_d[kc * 128:(kc + 1) * 128, ms])
                    nc.tensor.matmul(px[:], wsl[:], h2_3[kc][:],
                                     start=(kc == 0), stop=(kc == 3))
                gpre = vpool.tile([128, 4], dt.float32, name="gpre")
                nc.vector.tensor_reduce(
                    gpre[:], px[:].rearrange("p (b n) -> p b n", n=128),
                    axis=AX.X, op=AluOpType.max)
                nc.scalar.activation(gall[:, mc * 4:(mc + 1) * 4], gpre[:],
                                     AF.Identity, bias=b3_3t[:, mc:mc + 1])
            for mc in range(4):
                ms = slice(mc * 128, (mc + 1) * 128)
                pf = ps.tile([128, 4], dt.float32, name="acc")
                for kc in range(8):
                    wsl = wpool.tile([128, 128], dt.float32r, name="wsl")
                    dma(wsl[:], f1w_d[kc * 128:(kc + 1) * 128, ms])
                    nc.tensor.matmul(pf[:], wsl[:],
                                     gall[:, kc * 4:(kc + 1) * 4],
                                     start=(kc == 0), stop=(kc == 7))
                nc.scalar.activation(hf1[:, mc * 4:(mc + 1) * 4], pf[:],
                                     AF.Relu, bias=f1bt[:, mc:mc + 1])
            for mc in range(2):
                ms = slice(mc * 128, (mc + 1) * 128)
                pf = ps.tile([128, 4], dt.float32, name="acc")
                for kc in range(4):
                    wsl = wpool.tile([128, 128], dt.float32r, name="wsl")
                    dma(wsl[:], f2w_d[kc * 128:(kc + 1) * 128, ms])
                    nc.tensor.matmul(pf[:], wsl[:],
                                     hf1[:, kc * 4:(kc + 1) * 4],
                                     start=(kc == 0), stop=(kc == 3))
                nc.scalar.activation(hf2[:, mc * 4:(mc + 1) * 4], pf[:],
                                     AF.Relu, bias=f2bt[:, mc:mc + 1])
            pf = ps.tile([40, 4], dt.float32, name="acc")
            for kc in range(2):
                nc.tensor.matmul(pf[:], f3w_t[kc][:],
                                 hf2[:, kc * 4:(kc + 1) * 4],
                                 start=(kc == 0), stop=(kc == 1))
            nc.scalar.activation(outsb[:], pf[:], AF.Identity, bias=f3bt[:])
            for b in range(BPC):
                dma(out_d[b:b + 1, :], outsb[:, b:b + 1])

    nc.compile()
    return nc


def _prep_weights(inputs):
    f32 = np.float32
    w = {}
    w["ones"] = np.ones((1, 512), f32)
    w["w1t6"] = np.ascontiguousarray(inputs["sa1_w1"].T, f32)
    w["w1x1"] = np.ascontiguousarray(inputs["sa1_w1"][:, 0:3].T, f32)
    w["b1_1"] = inputs["sa1_b1"].reshape(64, 1).astype(f32)
    w["w2t1"] = np.ascontiguousarray(inputs["sa1_w2"].T, f32)
    w["b2_1"] = inputs["sa1_b2"].reshape(64, 1).astype(f32)
    w["w3t1"] = np.ascontiguousarray(inputs["sa1_w3"].T, f32)
    w["b3_1"] = inputs["sa1_b3"].reshape(128, 1).astype(f32)
    w["w1x2"] = np.ascontiguousarray(inputs["sa2_w1"][:, 0:3].T, f32)
    w["w1p2"] = np.ascontiguousarray(inputs["sa2_w1"][:, 3:].T, f32)
    w["b1_2"] = inputs["sa2_b1"].reshape(128, 1).astype(f32)
    w["w2t2"] = np.ascontiguousarray(inputs["sa2_w2"].T, f32)
    w["b2_2"] = inputs["sa2_b2"].reshape(128, 1).astype(f32)
    w["w3t2"] = np.ascontiguousarray(inputs["sa2_w3"].T, f32)
    w["b3_2"] = np.ascontiguousarray(
        inputs["sa2_b3"].reshape(2, 128).T, f32)
    w["w1p3"] = np.ascontiguousarray(inputs["sa3_w1"][:, 3:].T, f32)
    w["b1_3"] = np.ascontiguousarray(
        inputs["sa3_b1"].reshape(2, 128).T, f32)
    w["w2t3"] = np.ascontiguousarray(inputs["sa3_w2"].T, f32)
    w["b2_3"] = np.ascontiguousarray(
        inputs["sa3_b2"].reshape(4, 128).T, f32)
    w["w3t3"] = np.ascontiguousarray(inputs["sa3_w3"].T, f32)
    w["b3_3"] = np.ascontiguousarray(
        inputs["sa3_b3"].reshape(8, 128).T, f32)
    w["f1w"] = np.ascontiguousarray(inputs["fc1_w"].T, f32)
    w["f1b"] = np.ascontiguousarray(
        inputs["fc1_b"].reshape(4, 128).T, f32)
    w["f2w"] = np.ascontiguousarray(inputs["fc2_w"].T, f32)
    w["f2b"] = np.ascontiguousarray(
        inputs["fc2_b"].reshape(2, 128).T, f32)
    w["f3w"] = np.ascontiguousarray(inputs["fc3_w"].T, f32)
    w["f3b"] = inputs["fc3_b"].reshape(40, 1).astype(f32)
    w["eye"] = np.eye(128, dtype=f32)
    e64 = np.eye(64, dtype=np.float16)
    w["eyestk"] = np.ascontiguousarray(np.vstack([e64, e64]))
    return w


def kernel(**inputs):
    global last_exec_time_ns
    if "nc" not in _cache:
        _cache["nc"] = build_program()
    nc = _cache["nc"]
    w = _prep_weights(inputs)
    x = np.ascontiguousarray(inputs["x"], np.float32)
    in_maps = []
    for c in range(NCORES):
        m = dict(w)
        m["x"] = np.ascontiguousarray(x[c * BPC:(c + 1) * BPC])
        in_maps.append(m)
    import os
    trace = bool(int(os.environ.get("KERNEL_TRACE", "0")))
    try:
        res = bass_utils.run_bass_kernel_spmd(
            nc, in_maps, list(range(NCORES)), trace=trace)
    except ModuleNotFoundError:
        res = bass_utils.run_bass_kernel_spmd(
            nc, in_maps, list(range(NCORES)), trace=False)
    last_exec_time_ns = getattr(res, "exec_time_ns", None)
    if last_exec_time_ns is None:
        import time as _t
        t0 = _t.perf_counter()
        bass_utils.run_bass_kernel_spmd(
            nc, in_maps, list(range(NCORES)), trace=False)
        last_exec_time_ns = int((_t.perf_counter() - t0) * 1e9)
    out = np.concatenate([res.results[c]["out"] for c in range(NCORES)], 0)
    return out.astype(np.float32)



# revision 2
# speedup vs baseline: 1.2358x; 1.2358x over previous
"""PointNet2 classifier Trainium2 kernel.

Contract: kernel(**inputs) takes full unsharded inputs (x [32,6,4096] plus
weights), shards batch over 8 neuron cores (4 batches/core), runs a Bass
program per core, returns [32,40] fp32.

Per-core program (per batch):
  SA1: scores s=2*q.x - ys (fp32 PE matmul, zero topk flips vs reference),
       DVE top-32 (max/max_index/match_replace rounds), A-matrix trick:
       A = W1@[xyz;pts] stored hi/lo fp16 (22-bit), dma_gather(transpose),
       h1 = relu(Ahi+Alo - C + b1), convs 64->64->128 in fp32r, fused
       max-over-k from PSUM.
  SA2: same with N=512,P=128,k=64, A2 channels 128 (hi/lo 512B rows).
  SA3 + FC head: batched over the 4 batches (512 cols).
"""
import sys
sys.path.insert(0, '/opt/trn_rl_repo')
import numpy as np
from concourse import bass, bacc, tile, bass_utils
from concourse.alu_op_type import AluOpType

mybir = bass.mybir
dt = mybir.dt
AF = mybir.ActivationFunctionType
AX = mybir.AxisListType

BPC = 4
N = 4096
NEG = -3.0e38
NCORES = 8

_cache = {}
last_exec_time_ns = None


def build_program():
    nc = bacc.Bacc("TRN2", target_bir_lowering=False, debug=False,
                   num_devices=1)

    x_d = nc.dram_tensor("x", [BPC, 6, N], dt.float32, kind="ExternalInput")
    ones_d = nc.dram_tensor("ones", [1, 512], dt.float32,
                            kind="ExternalInput")

    def win(name, shape):
        return nc.dram_tensor(name, shape, dt.float32, kind="ExternalInput")

    w1t6_d = win("w1t6", [6, 64])
    w1x1_d = win("w1x1", [3, 64])
    b1_1_d = win("b1_1", [64, 1])
    w2t1_d = win("w2t1", [64, 64])
    b2_1_d = win("b2_1", [64, 1])
    w3t1_d = win("w3t1", [64, 128])
    b3_1_d = win("b3_1", [128, 1])
    w1x2_d = win("w1x2", [3, 128])
    w1p2_d = win("w1p2", [128, 128])
    b1_2_d = win("b1_2", [128, 1])
    w2t2_d = win("w2t2", [128, 128])
    b2_2_d = win("b2_2", [128, 1])
    w3t2_d = win("w3t2", [128, 256])
    b3_2_d = win("b3_2", [128, 2])
    w1p3_d = win("w1p3", [256, 256])
    b1_3_d = win("b1_3", [128, 2])
    w2t3_d = win("w2t3", [256, 512])
    b2_3_d = win("b2_3", [128, 4])
    w3t3_d = win("w3t3", [512, 1024])
    b3_3_d = win("b3_3", [128, 8])
    f1w_d = win("f1w", [1024, 512])
    f1b_d = win("f1b", [128, 4])
    f2w_d = win("f2w", [512, 256])
    f2b_d = win("f2b", [128, 2])
    f3w_d = win("f3w", [256, 40])
    f3b_d = win("f3b", [40, 1])

    eye_d = nc.dram_tensor("eye", [128, 128], dt.float32,
                           kind="ExternalInput")
    eyestk_d = nc.dram_tensor("eyestk", [128, 64], dt.float16,
                              kind="ExternalInput")
    out_d = nc.dram_tensor("out", [BPC, 40], dt.float32,
                           kind="ExternalOutput")

    with tile.TileContext(nc) as tc, \
         tc.tile_pool(name="persist", bufs=1) as pp:
        # ---------------- persistent weight tiles ----------------
        w1t6_t = pp.tile([6, 64], dt.float32r, name="w1t6_t")
        w1x1_t = pp.tile([3, 64], dt.float32, name="w1x1_t")
        b1_1t = pp.tile([64, 1], dt.float32, name="b1_1t")
        w2t1_t = pp.tile([64, 64], dt.float32r, name="w2t1_t")
        b2_1t = pp.tile([64, 1], dt.float32, name="b2_1t")
        w3t1_t = pp.tile([64, 128], dt.float32r, name="w3t1_t")
        b3_1t = pp.tile([128, 1], dt.float32, name="b3_1t")
        w1x2_r = pp.tile([3, 128], dt.float32r, name="w1x2_r")
        w1x2_f = pp.tile([3, 128], dt.float32, name="w1x2_f")
        w1p2_t = pp.tile([128, 128], dt.float32r, name="w1p2_t")
        b1_2t = pp.tile([128, 1], dt.float32, name="b1_2t")
        w2t2_t = pp.tile([128, 128], dt.float32r, name="w2t2_t")
        b2_2t = pp.tile([128, 1], dt.float32, name="b2_2t")
        w3t2_t = pp.tile([128, 256], dt.float32r, name="w3t2_t")
        b3_2t = pp.tile([128, 2], dt.float32, name="b3_2t")
        w1p3_t = [pp.tile([128, 256], dt.float32r, name=f"w1p3_{k}")
                  for k in range(2)]
        b1_3t = pp.tile([128, 2], dt.float32, name="b1_3t")
        w2t3_t = [pp.tile([128, 512], dt.float32r, name=f"w2t3_{k}")
                  for k in range(2)]
        b2_3t = pp.tile([128, 4], dt.float32, name="b2_3t")
        b3_3t = pp.tile([128, 8], dt.float32, name="b3_3t")
        f1bt = pp.tile([128, 4], dt.float32, name="f1bt")
        f2bt = pp.tile([128, 2], dt.float32, name="f2bt")
        f3w_t = [pp.tile([128, 40], dt.float32r, name=f"f3w_{k}")
                 for k in range(2)]
        f3bt = pp.tile([40, 1], dt.float32, name="f3bt")
        ones_t = pp.tile([1, 512], dt.float32, name="ones_t")
        ones3_t = pp.tile([3, 1], dt.float32, name="ones3_t")
        eye_t = pp.tile([128, 128], dt.float32, name="eye_t")
        eyestk_t = pp.tile([128, 64], dt.float16, name="eyestk_t")

        dma = nc.gpsimd.dma_start
        dma(w1t6_t[:], w1t6_d[:])
        dma(w1x1_t[:], w1x1_d[:])
        dma(b1_1t[:], b1_1_d[:])
        dma(w2t1_t[:], w2t1_d[:])
        dma(b2_1t[:], b2_1_d[:])
        dma(w3t1_t[:], w3t1_d[:])
        dma(b3_1t[:], b3_1_d[:])
        dma(w1x2_r[:], w1x2_d[:])
        dma(w1x2_f[:], w1x2_d[:])
        dma(w1p2_t[:], w1p2_d[:])
        dma(b1_2t[:], b1_2_d[:])
        dma(w2t2_t[:], w2t2_d[:])
        dma(b2_2t[:], b2_2_d[:])
        dma(w3t2_t[:], w3t2_d[:])
        dma(b3_2t[:], b3_2_d[:])
        for k in range(2):
            dma(w1p3_t[k][:], w1p3_d[k * 128:(k + 1) * 128, :])
            dma(w2t3_t[k][:], w2t3_d[k * 128:(k + 1) * 128, :])
            dma(f3w_t[k][:], f3w_d[k * 128:(k + 1) * 128, :])
        dma(b1_3t[:], b1_3_d[:])
        dma(b2_3t[:], b2_3_d[:])
        dma(b3_3t[:], b3_3_d[:])
        dma(f1bt[:], f1b_d[:])
        dma(f2bt[:], f2b_d[:])
        dma(f3bt[:], f3b_d[:])
        dma(ones_t[:], ones_d[:])
        dma(ones3_t[:], ones_d[0:1, 0:3])
        dma(eye_t[:], eye_d[:])
        dma(eyestk_t[:], eyestk_d[:])

        # ---------------- persistent working tiles ----------------
        caug = pp.tile([4, N], dt.float32, name="caug")   # [x;y;z;-ys]
        qaug = pp.tile([4, 512], dt.float32, name="qaug")  # [2x;2y;2z;1]
        xyz1_r = pp.tile([3, 512], dt.float32r, name="xyz1_r")
        C1 = pp.tile([64, 512], dt.float32, name="C1")
        C2 = pp.tile([128, 128], dt.float32, name="C2")
        p1pre = pp.tile([128, 512], dt.float32, name="p1pre")
        p1 = pp.tile([128, 512], dt.float32r, name="p1")
        p2pre_h = pp.tile([128, 128], dt.float32, name="p2pre_h")
        p2pre_l = pp.tile([128, 128], dt.float32, name="p2pre_l")
        p2all_h = pp.tile([128, 512], dt.float32r, name="p2all_h")
        p2all_l = pp.tile([128, 512], dt.float32r, name="p2all_l")
        A1sb = pp.tile([64, 4096], dt.float32, name="A1sb")
        A2sb = pp.tile([128, 512], dt.float32, name="A2sb")
        idxw = pp.tile([128, 1024], dt.int16, name="idxw")
        S2 = pp.tile([128, 512], dt.float32, name="S2")
        gall = pp.tile([128, 32], dt.float32r, name="gall")
        hf1 = pp.tile([128, 16], dt.float32r, name="hf1")
        hf2 = pp.tile([128, 8], dt.float32r, name="hf2")
        outsb = pp.tile([40, 4], dt.float32, name="outsb")

        with tc.tile_pool(name="ps", bufs=8, space="PSUM") as ps, \
             tc.tile_pool(name="spool", bufs=2) as spool, \
             tc.tile_pool(name="apool", bufs=3) as apool, \
             tc.tile_pool(name="vpool", bufs=2) as vpool, \
             tc.tile_pool(name="ipool", bufs=2) as ipool, \
             tc.tile_pool(name="hpool", bufs=2) as hpool, \
             tc.tile_pool(name="wpool", bufs=4) as wpool, \
             tc.tile_pool(name="gpool", bufs=2) as gpool:

            for b in range(BPC):
                # ---- load xyz / features ----
                dma(caug[0:3, :], x_d[b, 0:3, :])
                dma(qaug[3:4, :], ones_d[:])
                dma(xyz1_r[:], x_d[b, 0:3, 0:512])

                # ---- squares and ys (caug row 3 = -ys) ----
                for ns in range(8):
                    sqc = vpool.tile([3, 512], dt.float32, name="sqc")
                    nc.scalar.activation(sqc[:],
                                         caug[0:3, ns * 512:(ns + 1) * 512],
                                         AF.Square)
                    ys_ps = ps.tile([1, 512], dt.float32, name="acc")
                    nc.tensor.matmul(ys_ps[:], ones3_t[:], sqc[:],
                                     start=True, stop=True)
                    ys_sb = vpool.tile([1, 512], dt.float32, name="ys_sb")
                    nc.scalar.activation(ys_sb[:], ys_ps[:], AF.Copy,
                                         scale=-1.0)
                    dma(caug[3:4, ns * 512:(ns + 1) * 512], ys_sb[:])
                nc.scalar.activation(qaug[0:3, :], caug[0:3, 0:512],
                                     AF.Copy, scale=2.0)

                # ---- A1 = W1 @ [xyz;pts], channel-major fp32 [64, 4096] ----
                for cg in range(8):
                    xc = apool.tile([6, 512], dt.float32r, name="xc")
                    dma(xc[:], x_d[b, :, cg * 512:(cg + 1) * 512])
                    a_ps = ps.tile([64, 512], dt.float32, name="acc")
                    nc.tensor.matmul(a_ps[:], w1t6_t[:], xc[:],
                                     start=True, stop=True)
                    nc.scalar.activation(A1sb[:, cg * 512:(cg + 1) * 512],
                                         a_ps[:], AF.Copy)

                # ---- C1 = W1[:, :3] @ q_xyz (fp32) ----
                c_ps = ps.tile([64, 512], dt.float32, name="acc")
                nc.tensor.matmul(c_ps[:], w1x1_t[:], caug[0:3, 0:512],
                                 start=True, stop=True)
                nc.scalar.activation(C1[:], c_ps[:], AF.Copy)

                # ---- SA1 scores + top-32 per 128-query chunk ----
                for qc in range(4):
                    S = spool.tile([128, N], dt.float32, name="S")
                    for ns in range(8):
                        s_ps = ps.tile([128, 512], dt.float32, name="acc")
                        nc.tensor.matmul(
                            s_ps[:], qaug[:, qc * 128:(qc + 1) * 128],
                            caug[:, ns * 512:(ns + 1) * 512],
                            start=True, stop=True)
                        nc.scalar.activation(S[:, ns * 512:(ns + 1) * 512],
                                             s_ps[:], AF.Copy)
                    idx32 = ipool.tile([128, 32], dt.uint32, name="idx32")
                    for r in range(4):
                        v8 = vpool.tile([128, 8], dt.float32, name="v8")
                        nc.vector.max(v8[:], S[:])
                        nc.vector.max_index(idx32[:, r * 8:(r + 1) * 8],
                                            v8[:], S[:])
                        if r < 3:
                            nc.vector.match_replace(S[:], v8[:], S[:], NEG)
                    idxf = ipool.tile([128, 32], dt.float32, name="idxf")
                    nc.vector.tensor_copy(idxf[:], idx32[:])
                    tp_ps = ps.tile([32, 128], dt.float32, name="acc")
                    nc.tensor.transpose(tp_ps[:], idxf[:], eye_t[:])
                    tpf = ipool.tile([32, 128], dt.float32, name="tpf")
                    nc.scalar.activation(tpf[:], tp_ps[:], AF.Copy)
                    # wrap: flat i = q*32+r -> idxw[r%16, 2q + r//16]
                    for h in range(2):
                        sel_ps = ps.tile([16, 128], dt.float32, name="acc")
                        nc.tensor.matmul(
                            sel_ps[:], eye_t[0:32, 16 * h:16 * h + 16],
                            tpf[:], start=True, stop=True)
                        dst = idxw[0:16, qc * 256:(qc + 1) * 256].rearrange(
                            "p (q h) -> p q h", h=2)[:, :, h:h + 1]
                        nc.vector.tensor_copy(dst, sel_ps[:].unsqueeze(2))

                # ---- replicate wrap channels 16 -> 128, gather A1 rows ----
                dma(idxw[16:32, 0:1024], idxw[0:16, 0:1024])
                dma(idxw[32:64, 0:1024], idxw[0:32, 0:1024])
                dma(idxw[64:128, 0:1024], idxw[0:64, 0:1024])
                # ---- SA1: per-qc ap_gather + conv (8 chunks of 512) ----
                for qc in range(4):
                    G1f = gpool.tile([64, 4096], dt.float32, name="G1f")
                    nc.gpsimd.ap_gather(
                        G1f[:].unsqueeze(2), A1sb[:].unsqueeze(2),
                        idxw[0:64, qc * 256:(qc + 1) * 256],
                        channels=64, num_elems=4096, d=1, num_idxs=4096)
                    for ch in range(8):
                        cs = slice(ch * 512, (ch + 1) * 512)
                        q0, q1 = qc * 128 + ch * 16, qc * 128 + ch * 16 + 16
                        h1s = hpool.tile([64, 512], dt.float32, name="h1s")
                        cb = C1[:, q0:q1].unsqueeze(2).broadcast_to(
                            [64, 16, 32])
                        nc.vector.tensor_sub(
                            h1s[:].rearrange("p (q k) -> p q k", k=32),
                            G1f[:, cs].rearrange("p (q k) -> p q k", k=32),
                            cb)
                        h1 = hpool.tile([64, 512], dt.float32r, name="h1")
                        nc.scalar.activation(h1[:], h1s[:], AF.Relu,
                                             bias=b1_1t[:])
                        p2_ps = ps.tile([64, 512], dt.float32, name="acc")
                        nc.tensor.matmul(p2_ps[:], w2t1_t[:], h1[:],
                                         start=True, stop=True)
                        h2 = hpool.tile([64, 512], dt.float32r, name="h2")
                        nc.scalar.activation(h2[:], p2_ps[:], AF.Relu,
                                             bias=b2_1t[:])
                        p3_ps = ps.tile([128, 512], dt.float32, name="acc")
                        nc.tensor.matmul(p3_ps[:], w3t1_t[:], h2[:],
                                         start=True, stop=True)
                        nc.vector.tensor_reduce(
                            p1pre[:, q0:q1],
                            p3_ps[:].rearrange("p (q k) -> p q k", k=32),
                            axis=AX.X, op=AluOpType.max)
                nc.scalar.activation(p1[:], p1pre[:], AF.Identity,
                                     bias=b3_1t[:])

                # ---- A2 = W1_2 @ [xyz1;p1], channel-major fp32 [128, 512] --
                a2_ps = ps.tile([128, 512], dt.float32, name="acc")
                nc.tensor.matmul(a2_ps[:], w1x2_r[:], xyz1_r[:],
                                 start=True, stop=False)
                nc.tensor.matmul(a2_ps[:], w1p2_t[:], p1[:],
                                 start=False, stop=True)
                nc.scalar.activation(A2sb[:], a2_ps[:], AF.Copy)

                # ---- C2 (fp32) ----
                c2_ps = ps.tile([128, 128], dt.float32, name="acc")
                nc.tensor.matmul(c2_ps[:], w1x2_f[:], caug[0:3, 0:128],
                                 start=True, stop=True)
                nc.scalar.activation(C2[:], c2_ps[:], AF.Copy)

                # ---- SA2 scores (reuse qaug/caug slices) + top-64 ----
                s2_ps = ps.tile([128, 512], dt.float32, name="acc")
                nc.tensor.matmul(s2_ps[:], qaug[:, 0:128], caug[:, 0:512],
                                 start=True, stop=True)
                nc.scalar.activation(S2[:, 0:512], s2_ps[:], AF.Copy)
                idx64 = ipool.tile([128, 64], dt.uint32, name="idx64")
                for r in range(8):
                    v8 = vpool.tile([128, 8], dt.float32, name="v8b")
                    nc.vector.max(v8[:], S2[:, 0:512])
                    nc.vector.max_index(idx64[:, r * 8:(r + 1) * 8], v8[:],
                                        S2[:, 0:512])
                    if r < 7:
                        nc.vector.match_replace(S2[:, 0:512], v8[:],
                                                S2[:, 0:512], NEG)
                idxf2 = ipool.tile([128, 64], dt.float32, name="idxf2")
                nc.vector.tensor_copy(idxf2[:], idx64[:])
                tp2_ps = ps.tile([64, 128], dt.float32, name="acc")
                nc.tensor.transpose(tp2_ps[:], idxf2[:], eye_t[:])
                tpf2 = ipool.tile([64, 128], dt.float32, name="tpf2")
                nc.scalar.activation(tpf2[:], tp2_ps[:], AF.Copy)
                # wrap: flat i = q*64+r -> idxw[r%16, 4q + r//16]
                for h in range(4):
                    sel_ps = ps.tile([16, 128], dt.float32, name="acc")
                    nc.tensor.matmul(
                        sel_ps[:], eye_t[0:64, 16 * h:16 * h + 16],
                        tpf2[:], start=True, stop=True)
                    dst = idxw[0:16, 0:512].rearrange(
                        "p (q h) -> p q h", h=4)[:, :, h:h + 1]
                    nc.vector.tensor_copy(dst, sel_ps[:].unsqueeze(2))
                dma(idxw[16:32, 0:512], idxw[0:16, 0:512])
                dma(idxw[32:64, 0:512], idxw[0:32, 0:512])
                dma(idxw[64:128, 0:512], idxw[0:64, 0:512])
                # ---- SA2: chunked ap_gather + conv (4x4 chunks of 512) ----
                for c4 in range(4):
                    G2f = gpool.tile([128, 2048], dt.float32, name="G2f")
                    nc.gpsimd.ap_gather(
                        G2f[:].unsqueeze(2), A2sb[:].unsqueeze(2),
                        idxw[0:128, c4 * 128:(c4 + 1) * 128],
                        channels=128, num_elems=512, d=1, num_idxs=2048)
                    for ci in range(4):
                        cs = slice(ci * 512, (ci + 1) * 512)
                        q0 = c4 * 32 + ci * 8
                        q1 = q0 + 8
                        h1s = hpool.tile([128, 512], dt.float32, name="h1s")
                        cb = C2[:, q0:q1].unsqueeze(2).broadcast_to(
                            [128, 8, 64])
                        nc.vector.tensor_sub(
                            h1s[:].rearrange("p (q k) -> p q k", k=64),
                            G2f[:, cs].rearrange("p (q k) -> p q k", k=64),
                            cb)
                        h1 = hpool.tile([128, 512], dt.float32r, name="h1")
                        nc.scalar.activation(h1[:], h1s[:], AF.Relu,
                                             bias=b1_2t[:])
                        p2_ps = ps.tile([128, 512], dt.float32, name="acc")
                        nc.tensor.matmul(p2_ps[:], w2t2_t[:], h1[:],
                                         start=True, stop=True)
                        h2 = hpool.tile([128, 512], dt.float32r, name="h2")
                        nc.scalar.activation(h2[:], p2_ps[:], AF.Relu,
                                             bias=b2_2t[:])
                        ph = ps.tile([128, 512], dt.float32, name="acc")
                        nc.tensor.matmul(ph[:], w3t2_t[:, 0:128], h2[:],
                                         start=True, stop=True)
                        nc.vector.tensor_reduce(
                            p2pre_h[:, q0:q1],
                            ph[:].rearrange("p (q k) -> p q k", k=64),
                            axis=AX.X, op=AluOpType.max)
                        pl = ps.tile([128, 512], dt.float32, name="acc")
                        nc.tensor.matmul(pl[:], w3t2_t[:, 128:256], h2[:],
                                         start=True, stop=True)
                        nc.vector.tensor_reduce(
                            p2pre_l[:, q0:q1],
                            pl[:].rearrange("p (q k) -> p q k", k=64),
                            axis=AX.X, op=AluOpType.max)
                bs = slice(b * 128, (b + 1) * 128)
                nc.scalar.activation(p2all_h[:, bs], p2pre_h[:], AF.Identity,
                                     bias=b3_2t[:, 0:1])
                nc.scalar.activation(p2all_l[:, bs], p2pre_l[:], AF.Identity,
                                     bias=b3_2t[:, 1:2])

            # ---------------- SA3 + FC head (batched, 512 cols) ----------
            h1_3 = [pp.tile([128, 512], dt.float32r, name=f"h1_3{k}")
                    for k in range(2)]
            h2_3 = [pp.tile([128, 512], dt.float32r, name=f"h2_3{k}")
                    for k in range(4)]
            for mc in range(2):
                ms = slice(mc * 128, (mc + 1) * 128)
                px = ps.tile([128, 512], dt.float32, name="acc")
                nc.tensor.matmul(px[:], w1p3_t[0][:, ms], p2all_h[:],
                                 start=True, stop=False)
                nc.tensor.matmul(px[:], w1p3_t[1][:, ms], p2all_l[:],
                                 start=False, stop=True)
                nc.scalar.activation(h1_3[mc][:], px[:], AF.Relu,
                                     bias=b1_3t[:, mc:mc + 1])
            for mc in range(4):
                ms = slice(mc * 128, (mc + 1) * 128)
                px = ps.tile([128, 512], dt.float32, name="acc")
                for kc in range(2):
                    nc.tensor.matmul(px[:], w2t3_t[kc][:, ms], h1_3[kc][:],
                                     start=(kc == 0), stop=(kc == 1))
                nc.scalar.activation(h2_3[mc][:], px[:], AF.Relu,
                                     bias=b2_3t[:, mc:mc + 1])
            for mc in range(8):
                ms = slice(mc * 128, (mc + 1) * 128)
                px = ps.tile([128, 512], dt.float32, name="acc")
                for kc in range(4):
                    wsl = wpool.tile([128, 128], dt.float32r, name="wsl")
                    dma(wsl[:], w3t3_d[kc * 128:(kc + 1) * 128, ms])
                    nc.tensor.matmul(px[:], wsl[:], h2_3[kc][:],
                                     start=(kc == 0), stop=(kc == 3))
                gpre = vpool.tile([128, 4], dt.float32, name="gpre")
                nc.vector.tensor_reduce(
                    gpre[:], px[:].rearrange("p (b n) -> p b n", n=128),
                    axis=AX.X, op=AluOpType.max)
                nc.scalar.activation(gall[:, mc * 4:(mc + 1) * 4], gpre[:],
                                     AF.Identity, bias=b3_3t[:, mc:mc + 1])
            for mc in range(4):
                ms = slice(mc * 128, (mc + 1) * 128)
                pf = ps.tile([128, 4], dt.float32, name="acc")
                for kc in range(8):
                    wsl = wpool.tile([128, 128], dt.float32r, name="wsl")
                    dma(wsl[:], f1w_d[kc * 128:(kc + 1) * 128, ms])
                    nc.tensor.matmul(pf[:], wsl[:],
                                     gall[:, kc * 4:(kc + 1) * 4],
                                     start=(kc == 0), stop=(kc == 7))
                nc.scalar.activation(hf1[:, mc * 4:(mc + 1) * 4], pf[:],
                                     AF.Relu, bias=f1bt[:, mc:mc + 1])
            for mc in range(2):
                ms = slice(mc * 128, (mc + 1) * 128)
                pf = ps.tile([128, 4], dt.float32, name="acc")
                for kc in range(4):
                    wsl = wpool.tile([128, 128], dt.float32r, name="wsl")
                    dma(wsl[:], f2w_d[kc * 128:(kc + 1) * 128, ms])
                    nc.tensor.matmul(pf[:], wsl[:],
                                     hf1[:, kc * 4:(kc + 1) * 4],
                                     start=(kc == 0), stop=(kc == 3))
                nc.scalar.activation(hf2[:, mc * 4:(mc + 1) * 4], pf[:],
                                     AF.Relu, bias=f2bt[:, mc:mc + 1])
            pf = ps.tile([40, 4], dt.float32, name="acc")
            for kc in range(2):
                nc.tensor.matmul(pf[:], f3w_t[kc][:],
                                 hf2[:, kc * 4:(kc + 1) * 4],
                                 start=(kc == 0), stop=(kc == 1))
            nc.scalar.activation(outsb[:], pf[:], AF.Identity, bias=f3bt[:])
            for b in range(BPC):
                dma(out_d[b:b + 1, :], outsb[:, b:b + 1])

    nc.compile()
    return nc


def _prep_weights(inputs):
    f32 = np.float32
    w = {}
    w["ones"] = np.ones((1, 512), f32)
    w["w1t6"] = np.ascontiguousarray(inputs["sa1_w1"].T, f32)
    w["w1x1"] = np.ascontiguousarray(inputs["sa1_w1"][:, 0:3].T, f32)
    w["b1_1"] = inputs["sa1_b1"].reshape(64, 1).astype(f32)
    w["w2t1"] = np.ascontiguousarray(inputs["sa1_w2"].T, f32)
    w["b2_1"] = inputs["sa1_b2"].reshape(64, 1).astype(f32)
    w["w3t1"] = np.ascontiguousarray(inputs["sa1_w3"].T, f32)
    w["b3_1"] = inputs["sa1_b3"].reshape(128, 1).astype(f32)
    w["w1x2"] = np.ascontiguousarray(inputs["sa2_w1"][:, 0:3].T, f32)
    w["w1p2"] = np.ascontiguousarray(inputs["sa2_w1"][:, 3:].T, f32)
    w["b1_2"] = inputs["sa2_b1"].reshape(128, 1).astype(f32)
    w["w2t2"] = np.ascontiguousarray(inputs["sa2_w2"].T, f32)
    w["b2_2"] = inputs["sa2_b2"].reshape(128, 1).astype(f32)
    w["w3t2"] = np.ascontiguousarray(inputs["sa2_w3"].T, f32)
    w["b3_2"] = np.ascontiguousarray(
        inputs["sa2_b3"].reshape(2, 128).T, f32)
    w["w1p3"] = np.ascontiguousarray(inputs["sa3_w1"][:, 3:].T, f32)
    w["b1_3"] = np.ascontiguousarray(
        inputs["sa3_b1"].reshape(2, 128).T, f32)
    w["w2t3"] = np.ascontiguousarray(inputs["sa3_w2"].T, f32)
    w["b2_3"] = np.ascontiguousarray(
        inputs["sa3_b2"].reshape(4, 128).T, f32)
    w["w3t3"] = np.ascontiguousarray(inputs["sa3_w3"].T, f32)
    w["b3_3"] = np.ascontiguousarray(
        inputs["sa3_b3"].reshape(8, 128).T, f32)
    w["f1w"] = np.ascontiguousarray(inputs["fc1_w"].T, f32)
    w["f1b"] = np.ascontiguousarray(
        inputs["fc1_b"].reshape(4, 128).T, f32)
    w["f2w"] = np.ascontiguousarray(inputs["fc2_w"].T, f32)
    w["f2b"] = np.ascontiguousarray(
        inputs["fc2_b"].reshape(2, 128).T, f32)
    w["f3w"] = np.ascontiguousarray(inputs["fc3_w"].T, f32)
    w["f3b"] = inputs["fc3_b"].reshape(40, 1).astype(f32)
    w["eye"] = np.eye(128, dtype=f32)
    e64 = np.eye(64, dtype=np.float16)
    w["eyestk"] = np.ascontiguousarray(np.vstack([e64, e64]))
    return w


def kernel(**inputs):
    global last_exec_time_ns
    if "nc" not in _cache:
        _cache["nc"] = build_program()
    nc = _cache["nc"]
    w = _prep_weights(inputs)
    x = np.ascontiguousarray(inputs["x"], np.float32)
    in_maps = []
    for c in range(NCORES):
        m = dict(w)
        m["x"] = np.ascontiguousarray(x[c * BPC:(c + 1) * BPC])
        in_maps.append(m)
    import os
    trace = bool(int(os.environ.get("KERNEL_TRACE", "0")))
    tdir = os.environ.get("KERNEL_TRACE_DIR") or None
    if tdir:
        os.makedirs(tdir, exist_ok=True)
    try:
        res = bass_utils.run_bass_kernel_spmd(
            nc, in_maps, list(range(NCORES)), trace=trace, tmpdir=tdir)
    except ModuleNotFoundError:
        res = bass_utils.run_bass_kernel_spmd(
            nc, in_maps, list(range(NCORES)), trace=False)
    last_exec_time_ns = getattr(res, "exec_time_ns", None)
    if last_exec_time_ns is None:
        import time as _t
        t0 = _t.perf_counter()
        bass_utils.run_bass_kernel_spmd(
            nc, in_maps, list(range(NCORES)), trace=False)
        last_exec_time_ns = int((_t.perf_counter() - t0) * 1e9)
    out = np.concatenate([res.results[c]["out"] for c in range(NCORES)], 0)
    return out.astype(np.float32)

